# revision 31
# baseline (speedup 1.0000x reference)
"""Trainium2 Bass kernel for FOAM embedding (GNN message passing).

Strategy (8 NeuronCores, SPMD, no collectives):
  - Edges sorted by edge_src; host partitions nodes into 8 contiguous
    ranges with balanced edge counts; packs each core's edges into
    exact-128-edge blocks with 8 node slots (slot 7 = split head whose
    tail continues in the next block's slot 0).
  - Device scatter: per block one matmul PSUM[basis, (m,slot)] =
    dij^T @ S with dij [128e, 128b] and S [128e, 72] = Y (x) onehot.
    S is m-major so the phase-3 moving operand (7 slots for fixed m)
    is contiguous in SBUF -- a strided rhs cripples PE streaming.
  - Dij is a hybrid: most block groups ship dense bf16 from host, some
    are built on device (gpsimd) as senc (x) rb outer products from
    factored 24-col inputs, trading HBM traffic for pool-engine time.
  - Phase 3 per 14-block tile: x/y matmuls vs wx/wy into a shared
    2-bank PSUM supertile, y staged to SBUF (Act), x*y on DVE into a
    per-chunk product tile; m-sums are chunk-batched adds; r0 + all
    xy outputs leave in ONE contiguous DMA per chunk.
  - Scatter supertiles and phase-3 tiles interleave 1:1 on the PE
    queue to keep it fed and the p-state high.
"""

import os
import sys

import numpy as np

for _p in ("/opt/trn_rl_repo", "/root/.axon_site/_ro/trn_rl_repo"):
    if os.path.isdir(_p) and _p not in sys.path:
        sys.path.insert(0, _p)

import ml_dtypes  # noqa: E402

# ---------------- problem constants (hardcoded per spec) ----------------
N_RADIAL = 8
N_SPEC = 16
ZMAX = 64
CUTOFF = 5.0
NCHAN = 128
NB = N_RADIAL * N_SPEC  # 128 basis
M9 = 9                  # real SH components up to l=2

NCORES = 8
P = 128                 # edges per block == partitions
NSLOT = 8               # 7 completed-node slots + 1 split-head slot
SC = NSLOT * M9         # 72 S columns per block (m-major: m*8+s)
TBLK = 14               # blocks per phase-3 tile (5m*14*7 = 490 <= 512)
CH = 42                 # blocks per chunk (3 supertiles = 3 p3 tiles)
PSG = 7                 # blocks per scatter PSUM bank (7*72 = 504)
NT = CH // TBLK         # phase-3 tiles per chunk (3)

# global group assignment (cycled): which PSG groups have their dij
# built on device (pool engine) vs shipped dense from host.
GASSIGN = ("ship", "ship", "ship", "ship", "pool", "pool")
# chunk 0 ships everything: device builds would gate the pipeline start
SHIP_FIRST_CHUNK = True
# engine that drains each scatter PSUM supertile (2 groups) to SBUF
RHO_COPY = ("act", "act", "act")
# engine for the chunk-batched m-sum adds, cycled per chunk
ADDS_ENG = ("dve", "dve", "pool")

# per-chunk output/product tile layout (cols, bf16):
#   [0:294)      r0 (m=0, slots 0..6 of each block)
#   [294:1176)   od: (t, l, 98) xy outputs
#   [1176:3822)  products: per tile t at 1176+882*t:
#                  [0:98) l0, [98:392) l1 m-planes, [392:882) l2
PLW = 3822
ODO = 294
PRO = 1176

BF16 = ml_dtypes.bfloat16

_COMPILED = {}
TRACE = False          # set True to capture an NTFF profile
LAST_RESULT = None     # BassKernelResults of the last kernel() call

_S3, _S5, _S15 = 3.0 ** 0.5, 5.0 ** 0.5, 15.0 ** 0.5
KM = np.array([1.0, _S3, _S3, _S3, _S15, _S15,
               0.5 * _S5, _S15, 0.5 * _S15], np.float32)


# ======================= host-side preprocessing =======================

def _partition_cores(edge_src, n_nodes):
    """Split nodes into NCORES contiguous ranges with ~equal edges."""
    es = np.asarray(edge_src, dtype=np.int64)
    E = es.shape[0]
    splits = [0]
    for c in range(1, NCORES):
        n = int(es[min((c * E) // NCORES, E - 1)])
        n = max(n, splits[-1])
        splits.append(n)
    splits.append(n_nodes)
    return splits


def _pack_core(deg, first_edge, nlo, nhi):
    """Pack nodes [nlo, nhi) into exact-128-edge blocks."""
    blocks = []
    slot_nodes = []
    n = nlo
    carry = None  # (node, e_start, cnt) continuation -> slot 0
    while n < nhi or carry is not None:
        cnts = [0] * NSLOT
        snode = [-1] * NSLOT
        cap = P
        e_start = None
        si = 0
        if carry is not None:
            node, es0, cnt = carry
            assert cnt <= cap, f"node {node} degree too large"
            e_start = es0
            cnts[0] = cnt
            snode[0] = node
            cap -= cnt
            si = 1
            carry = None
        while n < nhi and si < NSLOT - 1:
            d = int(deg[n])
            if d > cap:
                break
            if e_start is None:
                e_start = int(first_edge[n])
            cnts[si] = d
            snode[si] = n
            cap -= d
            si += 1
            n += 1
        if cap > 0 and n < nhi:
            # split head into slot 7 (tail continues next block slot 0)
            d = int(deg[n])
            take = min(d, cap)
            if e_start is None:
                e_start = int(first_edge[n])
            cnts[NSLOT - 1] = take
            cap -= take
            carry = (n, int(first_edge[n]) + take, d - take)
            n += 1
        if e_start is None:
            e_start = int(first_edge[min(n, nhi - 1)])
        blocks.append((e_start, P - cap, cnts))
        slot_nodes.append(snode)
    return blocks, np.asarray(slot_nodes, np.int64)


def _chunk_plan(B):
    chs = []
    r = B
    while r > 0:
        c = min(CH, r)
        chs.append(c)
        r -= c
    plans = []
    for ci, ch in enumerate(chs):
        ngrp = (ch + PSG - 1) // PSG
        built, ship = [], []
        beng = []
        for g in range(ngrp):
            k0, k1 = g * PSG, min((g + 1) * PSG, ch)
            a = GASSIGN[g % len(GASSIGN)]
            if ci == 0 and SHIP_FIRST_CHUNK:
                a = "ship"
            if a == "ship":
                ship.extend(range(k0, k1))
            else:
                built.extend(range(k0, k1))
                beng.append((a, k0, k1))
        plans.append((ch, built, ship, beng))
    return chs, plans


def _build_core_inputs(blocks, B, ysw_e, senc_e, rb_e, dij_e, plans):
    """Build device DRAM arrays for one core.

    Returns sx [128, sum(ch*72+nbu*24)] bf16 (per-chunk [S | srb]),
    dijd [128, nship*128] bf16.
    """
    nb = len(blocks)
    eb = np.array([b[0] for b in blocks], np.int64)
    ne = np.array([b[1] for b in blocks], np.int64)
    cnts = np.array([b[2] for b in blocks], np.int64)  # [nb, 8]

    blk_of = np.repeat(np.arange(nb), ne)              # per packed edge
    row_of = np.arange(ne.sum()) - np.repeat(np.cumsum(ne) - ne, ne)
    edge_of = np.repeat(eb, ne) + row_of
    slot_of = np.concatenate([
        np.repeat(np.arange(NSLOT), cnts[k]) for k in range(nb)
    ]) if nb else np.zeros(0, np.int64)

    # S: m-major [B, P, 9, 8]
    S = np.zeros((B, P, M9, NSLOT), np.float32)
    S[blk_of, row_of, :, slot_of] = ysw_e[edge_of]

    SRB = np.zeros((B, P, N_SPEC + N_RADIAL), np.float32)
    SRB[blk_of, row_of, :N_SPEC] = senc_e[edge_of]
    SRB[blk_of, row_of, N_SPEC:] = rb_e[edge_of]

    sx_parts = []
    ship_idx = []
    c0 = 0
    for ch, built, ship, _ in plans:
        sp = np.ascontiguousarray(
            S[c0:c0 + ch].transpose(1, 0, 2, 3)).reshape(P, ch * SC)
        bp = np.ascontiguousarray(
            SRB[[c0 + k for k in built]].transpose(1, 0, 2)
        ).reshape(P, -1)
        sx_parts.append(sp)
        sx_parts.append(bp)
        ship_idx.extend(c0 + k for k in ship)
        c0 += ch
    sx = np.concatenate(sx_parts, axis=1).astype(BF16)

    ship_idx = np.array(ship_idx, np.int64)
    if len(ship_idx):
        D = np.zeros((len(ship_idx), P, NB), np.float32)
        mask = np.isin(blk_of, ship_idx)
        pos = np.full(B, -1, np.int64)
        pos[ship_idx] = np.arange(len(ship_idx))
        D[pos[blk_of[mask]], row_of[mask], :] = dij_e[edge_of[mask]]
        dijd = np.ascontiguousarray(
            D.transpose(1, 0, 2)).reshape(P, -1).astype(BF16)
    else:
        dijd = np.zeros((P, NB), BF16)
    return sx, dijd


def _perm_w(W):
    """Permute Dense weight rows from rs-order (r*16+s) to (s*8+r)."""
    W = np.asarray(W, np.float32)
    return np.ascontiguousarray(
        W.reshape(N_RADIAL, N_SPEC, -1).transpose(1, 0, 2).reshape(NB, -1)
    )


# ========================= device program =========================

def _build_program(B):
    import concourse.bacc as bacc
    import concourse.mybir as mybir
    import concourse.tile as tile
    from concourse.alu_op_type import AluOpType as alu

    fp32 = mybir.dt.float32
    bf16 = mybir.dt.bfloat16

    assert B % CH == 0, "B must be a multiple of CH for this schedule"
    chs, plans = _chunk_plan(B)
    nchunk = len(chs)
    # per-chunk offsets into sx / dijd (in columns / blocks)
    sx_off = [0]
    dij_off = [0]
    for ch, built, ship, _ in plans:
        sx_off.append(sx_off[-1] + ch * SC + len(built) * 24)
        dij_off.append(dij_off[-1] + len(ship))
    nship_tot = dij_off[-1]

    nc = bacc.Bacc("TRN2", target_bir_lowering=False, debug=False,
                   num_devices=NCORES)

    sx_d = nc.dram_tensor("sx", [P, sx_off[-1]], bf16,
                          kind="ExternalInput")
    dijd_d = nc.dram_tensor("dijd", [P, max(nship_tot, 1) * NB], bf16,
                            kind="ExternalInput")
    wx_d = nc.dram_tensor("wx", [P, 3 * NCHAN], bf16, kind="ExternalInput")
    wy_d = nc.dram_tensor("wy", [P, 3 * NCHAN], bf16, kind="ExternalInput")
    out_d = nc.dram_tensor("out", [P, nchunk * 1176], bf16,
                           kind="ExternalOutput")

    with tile.TileContext(nc) as tc:
        with (
            tc.tile_pool(name="const", bufs=1) as cpool,
            tc.tile_pool(name="chunk", bufs=3) as ckpool,
            tc.tile_pool(name="big", bufs=3) as bigpool,
            tc.tile_pool(name="work", bufs=3) as wkpool,
            tc.tile_pool(name="out", bufs=3) as opool,
            tc.tile_pool(name="ps_sc", bufs=2, space="PSUM") as pssc,
            tc.tile_pool(name="ps_xy", bufs=2, space="PSUM") as psxy,
        ):
            wx = cpool.tile([P, 3 * NCHAN], bf16, tag="wx")
            wy = cpool.tile([P, 3 * NCHAN], bf16, tag="wy")
            nc.sync.dma_start(out=wx[:], in_=wx_d[:])
            nc.sync.dma_start(out=wy[:], in_=wy_d[:])

            # keep the PE clock ramping while the first chunk DMA lands
            dum = cpool.tile([P, NCHAN], bf16, tag="dum")
            nc.vector.memset(dum[:], 0.0)
            psdum = psxy.tile([P, 1024], fp32, tag="ps")
            for _ in range(20):
                nc.tensor.matmul(out=psdum[:, 0:NCHAN], lhsT=dum[:],
                                 rhs=dum[:], start=True, stop=True)

            rtiles = {}   # ci -> rhoi sbuf tile
            dtiles = {}   # ci -> (dij, sx)
            ptiles = {}   # ci -> pl output/product tile

            def dma_part(ci):
                ch, built, ship, beng = plans[ci]
                dij = ckpool.tile([P, CH * NB], bf16, tag="dij")
                ss0 = ckpool.tile([P, TBLK * SC], bf16, tag="ss0")
                ss1 = ckpool.tile([P, TBLK * SC], bf16, tag="ss1")
                ss2 = ckpool.tile([P, TBLK * SC], bf16, tag="ss2")
                sb = ckpool.tile([P, CH * 24], bf16, tag="sb")
                sss = (ss0, ss1, ss2)
                dtiles[ci] = (dij, sss, sb)
                o = sx_off[ci]
                for u in range(NT):
                    nc.sync.dma_start(
                        out=sss[u][:],
                        in_=sx_d[:, o + u * TBLK * SC:
                                 o + (u + 1) * TBLK * SC])
                nbu = len(built)
                if nbu:
                    nc.sync.dma_start(
                        out=sb[:, 0:nbu * 24],
                        in_=sx_d[:, o + ch * SC:o + ch * SC + nbu * 24])
                nsh = len(ship)
                if nsh:
                    od = dij_off[ci]
                    k0 = ship[0]
                    # separate DGE ring (gpsimd) so the dense-dij bulk
                    # transfer runs concurrently with the S-tile ring
                    nc.gpsimd.dma_start(
                        out=dij[:, k0 * NB:(k0 + nsh) * NB],
                        in_=dijd_d[:, od * NB:(od + nsh) * NB])

            def build_part(ci):
                ch, built, ship, beng = plans[ci]
                dij, sss, sb = dtiles[ci]
                if not built:
                    return
                cpos = {k: i for i, k in enumerate(built)}
                dv = dij[:].rearrange("p (k s r) -> p k s r",
                                      s=N_SPEC, r=N_RADIAL)
                sv = sb[:, 0:len(built) * 24].rearrange(
                    "p (k c) -> p k c", c=24)
                for eng, k0, k1 in beng:
                    n = k1 - k0
                    i0 = cpos[k0]
                    senc = sv[:, i0:i0 + n, 0:N_SPEC]
                    rb = sv[:, i0:i0 + n, N_SPEC:24]
                    out = dv[:, k0:k1]
                    in0 = senc.unsqueeze(3).broadcast_to(
                        [P, n, N_SPEC, N_RADIAL])
                    in1 = rb.unsqueeze(2).broadcast_to(
                        [P, n, N_SPEC, N_RADIAL])
                    nc.gpsimd.tensor_tensor(out=out, in0=in0, in1=in1,
                                            op=alu.mult)

            def scatter_super(ci, sup):
                ch = chs[ci]
                dij, sss, sb = dtiles[ci]
                rhoi = rtiles[ci]
                pst = pssc.tile([P, 1024], fp32, tag="psc")
                kbase = sup * 2 * PSG
                nblk = min(2 * PSG, ch - kbase)
                ss = sss[sup]
                for j in range(nblk):
                    k = kbase + j
                    colb = (j // PSG) * 512 + (j % PSG) * SC
                    nc.tensor.matmul(
                        out=pst[:, colb:colb + SC],
                        lhsT=dij[:, k * NB:(k + 1) * NB],
                        rhs=ss[:, j * SC:(j + 1) * SC],
                        start=True, stop=True,
                    )
                assert nblk == 2 * PSG
                eng = RHO_COPY[sup % len(RHO_COPY)]
                src = pst[:].rearrange("p (g q) -> p g q", g=2)[
                    :, :, 0:PSG * SC].rearrange(
                    "p g (k c) -> p g k c", c=SC)
                dst = rhoi[:, kbase * SC:(kbase + nblk) * SC].rearrange(
                    "p (g k c) -> p g k c", g=2, c=SC)
                if eng == "act":
                    nc.scalar.copy(out=dst, in_=src)
                else:
                    nc.vector.tensor_copy(out=dst, in_=src)

            def merges(ci):
                ch = chs[ci]
                rv = rtiles[ci][:].rearrange("p (k m sl) -> p k m sl",
                                             m=M9, sl=NSLOT)
                nc.gpsimd.tensor_tensor(
                    out=rv[:, 1:ch, :, 0],
                    in0=rv[:, 1:ch, :, 0],
                    in1=rv[:, 0:ch - 1, :, 7],
                    op=alu.add,
                )
                if ci > 0:
                    pch = chs[ci - 1]
                    prv = rtiles[ci - 1][:].rearrange(
                        "p (k m sl) -> p k m sl", m=M9, sl=NSLOT)
                    nc.gpsimd.tensor_tensor(
                        out=rv[:, 0:1, :, 0],
                        in0=rv[:, 0:1, :, 0],
                        in1=prv[:, pch - 1:pch, :, 7],
                        op=alu.add,
                    )

            def phase3_tile(ci, t):
                rv = rtiles[ci][:].rearrange("p (k m sl) -> p k m sl",
                                             m=M9, sl=NSLOT)
                kk = t * TBLK
                ns = TBLK * 7  # 98
                pl = ptiles[ci]
                for unit in range(2):  # 0: l0+l1, 1: l2
                    ps = psxy.tile([P, 1024], fp32, tag="ps")
                    if unit == 0:
                        mms = [(0, 0, 0), (1, 0, 98), (1, 1, 196),
                               (1, 2, 294)]
                    else:
                        mms = [(2, mi, mi * 98) for mi in range(5)]
                    for off, w in ((0, wx), (512, wy)):
                        for l, mi, col in mms:
                            wl = w[:, l * NCHAN:(l + 1) * NCHAN]
                            mov = rv[:, kk:kk + TBLK, l * l + mi, 0:7]
                            nc.tensor.matmul(
                                out=ps[:, off + col:off + col + ns],
                                lhsT=wl, rhs=mov, start=True, stop=True)
                    nx = 392 if unit == 0 else 490
                    ysb = wkpool.tile([P, 512], bf16, tag="ysb")
                    nc.scalar.copy(out=ysb[:, 0:nx],
                                   in_=ps[:, 512:512 + nx])
                    if unit == 0:
                        nc.vector.tensor_tensor(
                            out=pl[:, PRO + t * 882:PRO + t * 882 + 392],
                            in0=ps[:, 0:392], in1=ysb[:, 0:392],
                            op=alu.mult)
                    else:
                        nc.vector.tensor_tensor(
                            out=pl[:, PRO + t * 882 + 392:
                                   PRO + t * 882 + 882],
                            in0=ps[:, 0:490], in1=ysb[:, 0:490],
                            op=alu.mult)

            def chunk_tail(ci):
                # chunk-batched m-sums + r0 extract + single output DMA
                ch = chs[ci]
                nt = ch // TBLK
                pl = ptiles.pop(ci)
                rv = rtiles[ci][:].rearrange("p (k m sl) -> p k m sl",
                                             m=M9, sl=NSLOT)
                adde = ADDS_ENG[ci % len(ADDS_ENG)]
                ae = nc.vector if adde == "dve" else nc.gpsimd
                pv = pl[:, PRO:PRO + nt * 882].rearrange(
                    "p (t w) -> p t w", w=882)
                ov = pl[:, ODO:ODO + nt * 294].rearrange(
                    "p (t l s) -> p t l s", l=3, s=98)
                tmp = wkpool.tile([P, NT * 2 * 98], bf16, tag="tmp")
                t1 = tmp[:, 0:nt * 98].rearrange("p (t s) -> p t s", s=98)
                # l0 output: straight copy of the l0 products (4x mode)
                nc.vector.tensor_copy(out=ov[:, :, 0, :],
                                      in_=pv[:, :, 0:98])
                # l1: m0 + m1 + m2
                ae.tensor_tensor(out=t1, in0=pv[:, :, 196:294],
                                 in1=pv[:, :, 294:392], op=alu.add)
                ae.tensor_tensor(out=ov[:, :, 1, :], in0=t1,
                                 in1=pv[:, :, 98:196], op=alu.add)
                # l2: ((m1+m2)+(m3+m4)) + m0
                p2 = pv[:, :, 490:882].rearrange("p t (i s) -> p t i s",
                                                 i=2, s=196)
                t2 = tmp[:, 0:nt * 2 * 98].rearrange(
                    "p (t i s) -> p t i s", i=2, s=98)
                ae.tensor_tensor(out=t2, in0=p2[:, :, :, 0:98],
                                 in1=p2[:, :, :, 98:196], op=alu.add)
                ae.tensor_tensor(out=t1, in0=t2[:, :, 0, :],
                                 in1=t2[:, :, 1, :], op=alu.add)
                ae.tensor_tensor(out=ov[:, :, 2, :], in0=t1,
                                 in1=pv[:, :, 392:490], op=alu.add)
                # r0 = m=0 plane, slots 0..6
                nc.vector.tensor_copy(
                    out=pl[:, 0:ch * 7].rearrange("p (k s) -> p k s", s=7),
                    in_=rv[:, 0:ch, 0, 0:7],
                )
                nc.sync.dma_start(
                    out=out_d[:, ci * 1176:(ci + 1) * 1176],
                    in_=pl[:, 0:1176])

            # software pipeline: input DMA two chunks ahead; pool builds
            # one chunk ahead of scatter; phase-3 one chunk behind
            # scatter, interleaved supertile-by-tile on the PE queue.
            for ci in range(nchunk + 2):
                if ci < nchunk:
                    dma_part(ci)
                    build_part(ci)   # a full iteration before its scatter
                cs = ci - 1           # scatter chunk
                cp = ci - 2           # phase-3 chunk
                if 0 <= cs < nchunk:
                    rh = bigpool.tile([P, CH * SC], bf16, tag="rhoi")
                    rtiles[cs] = rh
                if 0 <= cp < nchunk:
                    plt = opool.tile([P, PLW], bf16, tag="pl")
                    ptiles[cp] = plt
                for u in range(NT):
                    if 0 <= cs < nchunk:
                        scatter_super(cs, u)
                    if 0 <= cp < nchunk:
                        phase3_tile(cp, u)
                if 0 <= cs < nchunk:
                    merges(cs)
                if 0 <= cp < nchunk:
                    chunk_tail(cp)

    nc.finalize()
    return nc


# ============================ entry point ============================

def kernel(**inputs):
    from concourse.bass_utils import run_bass_kernel_spmd

    dist = np.asarray(inputs["distances"], np.float32)
    vec = np.asarray(inputs["vec"], np.float32)
    switch = np.asarray(inputs["switch"], np.float32)
    st = np.asarray(inputs["species_table"], np.float32)
    species = np.asarray(inputs["species"], np.int64)
    esrc = np.asarray(inputs["edge_src"], np.int64)
    edst = np.asarray(inputs["edge_dst"], np.int64)
    N_NODES = species.shape[0]
    E = esrc.shape[0]

    deg = np.bincount(esrc, minlength=N_NODES)
    assert deg.max() <= P, "node degree exceeds 128"
    first_edge = np.searchsorted(esrc, np.arange(N_NODES + 1), side="left")
    splits = _partition_cores(esrc, N_NODES)

    # per-edge factors
    nvec = np.arange(1, N_RADIAL + 1, dtype=np.float32)
    rb_e = (np.sqrt(2.0 / CUTOFF) * np.sin(nvec[None, :] * (np.pi / CUTOFF)
                                           * dist[:, None]) / dist[:, None]
            * switch[:, None]).astype(np.float32)           # [E, 8]
    senc_e = st[species[edst]]                              # [E, 16]
    u = vec / dist[:, None]
    x, y, z = u[:, 0], u[:, 1], u[:, 2]
    ysw_e = (np.stack([
        np.ones_like(x), x, y, z, x * y, y * z,
        3.0 * z * z - 1.0, x * z, x * x - y * y,
    ], axis=-1) * KM[None, :]).astype(np.float32)

    cores = []
    maxb = 0
    for c in range(NCORES):
        blocks, slot_node = _pack_core(deg, first_edge,
                                       splits[c], splits[c + 1])
        cores.append((blocks, slot_node))
        maxb = max(maxb, len(blocks))
    B = ((maxb + CH - 1) // CH) * CH
    B7 = B * (NSLOT - 1)
    chs, plans = _chunk_plan(B)
    nchunk = len(chs)

    # dij for shipped blocks: match device build numerics (bf16 inputs)
    dij_e = (senc_e.astype(BF16).astype(np.float32)[:, :, None]
             * rb_e.astype(BF16).astype(np.float32)[:, None, :]
             ).reshape(E, NB)

    wx = np.empty((P, 3 * NCHAN), np.float32)
    wy = np.empty((P, 3 * NCHAN), np.float32)
    for l, key in enumerate(("W0", "W1", "W2")):
        Wp = _perm_w(inputs[key])
        wx[:, l * NCHAN:(l + 1) * NCHAN] = Wp[:, :NCHAN]
        wy[:, l * NCHAN:(l + 1) * NCHAN] = (
            Wp[:, NCHAN:] / np.sqrt(2 * l + 1.0))
    wx = wx.astype(BF16)
    wy = wy.astype(BF16)

    in_maps = []
    for c in range(NCORES):
        blocks, _ = cores[c]
        sx, dijd = _build_core_inputs(
            blocks, B, ysw_e, senc_e, rb_e, dij_e, plans)
        in_maps.append({"sx": sx, "dijd": dijd, "wx": wx, "wy": wy})

    if B not in _COMPILED:
        _COMPILED[B] = _build_program(B)
    nc = _COMPILED[B]

    res = run_bass_kernel_spmd(nc, in_maps, list(range(NCORES)),
                               trace=TRACE)
    global LAST_RESULT
    LAST_RESULT = res

    # ---------------- host assembly ----------------
    out = np.zeros((N_NODES, N_SPEC + NB + 3 * NCHAN), np.float32)
    out[:, :N_SPEC] = st[species]

    # device basis row of original index rs = r*16+s is dev = s*8+r
    r = np.arange(NB) // N_SPEC
    sidx = np.arange(NB) % N_SPEC
    dev_of_rs = sidx * N_RADIAL + r

    for c in range(NCORES):
        _, slot_node = cores[c]
        sn = np.full((B, NSLOT - 1), -1, np.int64)
        sn[:slot_node.shape[0]] = slot_node[:, :NSLOT - 1]
        sn = sn.reshape(-1)
        valid = sn >= 0
        nodes = sn[valid]
        slots = np.nonzero(valid)[0]
        o = np.asarray(res.results[c]["out"], np.float32)  # [128, nck*1176]
        ov = o.reshape(P, nchunk, 1176)
        r0 = ov[:, :, 0:294].reshape(P, B7)
        xyv = ov[:, :, 294:1176].reshape(P, nchunk, NT, 3, 98)
        out[nodes, N_SPEC:N_SPEC + NB] = r0[dev_of_rs][:, slots].T
        for l in range(3):
            xy_l = np.ascontiguousarray(
                xyv[:, :, :, l, :]).reshape(P, B7)
            out[nodes,
                N_SPEC + NB + l * NCHAN:N_SPEC + NB + (l + 1) * NCHAN] = (
                xy_l[:, slots].T)
    return out


# revision 32
# speedup vs baseline: 1.0919x; 1.0919x over previous
"""Trainium2 Bass kernel for FOAM embedding (GNN message passing).

Strategy (8 NeuronCores, SPMD, no collectives):
  - Edges sorted by edge_src; host partitions nodes into 8 contiguous
    ranges with balanced edge counts; packs each core's edges into
    exact-128-edge blocks with 8 node slots (slot 7 = split head whose
    tail continues in the next block's slot 0).
  - Device scatter: per block one matmul PSUM[basis, (m,slot)] =
    dij^T @ S with dij [128e, 128b] and S [128e, 72] = Y (x) onehot.
    S is m-major so the phase-3 moving operand (7 slots for fixed m)
    is contiguous in SBUF -- a strided rhs cripples PE streaming.
  - Dij is a hybrid: most block groups ship dense bf16 from host, some
    are built on device (gpsimd) as senc (x) rb outer products from
    factored 24-col inputs, trading HBM traffic for pool-engine time.
  - Phase 3 per 14-block tile: x/y matmuls vs wx/wy into a shared
    2-bank PSUM supertile, y staged to SBUF (Act), x*y on DVE into a
    per-chunk product tile; m-sums are chunk-batched adds; r0 + all
    xy outputs leave in ONE contiguous DMA per chunk.
  - Scatter supertiles and phase-3 tiles interleave 1:1 on the PE
    queue to keep it fed and the p-state high.
"""

import os
import sys

import numpy as np

for _p in ("/opt/trn_rl_repo", "/root/.axon_site/_ro/trn_rl_repo"):
    if os.path.isdir(_p) and _p not in sys.path:
        sys.path.insert(0, _p)

import ml_dtypes  # noqa: E402

# ---------------- problem constants (hardcoded per spec) ----------------
N_RADIAL = 8
N_SPEC = 16
ZMAX = 64
CUTOFF = 5.0
NCHAN = 128
NB = N_RADIAL * N_SPEC  # 128 basis
M9 = 9                  # real SH components up to l=2

NCORES = 8
P = 128                 # edges per block == partitions
NSLOT = 8               # 7 completed-node slots + 1 split-head slot
SC = NSLOT * M9         # 72 S columns per block (m-major: m*8+s)
TBLK = 14               # blocks per phase-3 tile (5m*14*7 = 490 <= 512)
CH = 42                 # blocks per chunk (3 supertiles = 3 p3 tiles)
PSG = 7                 # blocks per scatter PSUM bank (7*72 = 504)
NT = CH // TBLK         # phase-3 tiles per chunk (3)

# global group assignment (cycled): which PSG groups have their dij
# built on device (pool engine) vs shipped dense from host.
GASSIGN = ("ship", "ship", "ship", "ship", "pool", "pool")
# chunk 0 ships everything: device builds would gate the pipeline start
SHIP_FIRST_CHUNK = True
# engine that drains each scatter PSUM supertile (2 groups) to SBUF
RHO_COPY = ("act", "act", "act")
# engine for the chunk-batched m-sum adds, cycled per chunk
ADDS_ENG = ("dve", "dve", "pool")

# per-chunk output/product tile layout (cols, bf16):
#   [0:294)      r0 (m=0, slots 0..6 of each block)
#   [294:1176)   od: (t, l, 98) xy outputs
#   [1176:3822)  products: per tile t at 1176+882*t:
#                  [0:98) l0, [98:392) l1 m-planes, [392:882) l2
PLW = 3822
ODO = 294
PRO = 1176

BF16 = ml_dtypes.bfloat16

_COMPILED = {}
TRACE = False          # set True to capture an NTFF profile
LAST_RESULT = None     # BassKernelResults of the last kernel() call

_S3, _S5, _S15 = 3.0 ** 0.5, 5.0 ** 0.5, 15.0 ** 0.5
KM = np.array([1.0, _S3, _S3, _S3, _S15, _S15,
               0.5 * _S5, _S15, 0.5 * _S15], np.float32)


# ======================= host-side preprocessing =======================

def _partition_cores(edge_src, n_nodes):
    """Split nodes into NCORES contiguous ranges with ~equal edges."""
    es = np.asarray(edge_src, dtype=np.int64)
    E = es.shape[0]
    splits = [0]
    for c in range(1, NCORES):
        n = int(es[min((c * E) // NCORES, E - 1)])
        n = max(n, splits[-1])
        splits.append(n)
    splits.append(n_nodes)
    return splits


def _pack_core(deg, first_edge, nlo, nhi):
    """Pack nodes [nlo, nhi) into exact-128-edge blocks."""
    blocks = []
    slot_nodes = []
    n = nlo
    carry = None  # (node, e_start, cnt) continuation -> slot 0
    while n < nhi or carry is not None:
        cnts = [0] * NSLOT
        snode = [-1] * NSLOT
        cap = P
        e_start = None
        si = 0
        if carry is not None:
            node, es0, cnt = carry
            assert cnt <= cap, f"node {node} degree too large"
            e_start = es0
            cnts[0] = cnt
            snode[0] = node
            cap -= cnt
            si = 1
            carry = None
        while n < nhi and si < NSLOT - 1:
            d = int(deg[n])
            if d > cap:
                break
            if e_start is None:
                e_start = int(first_edge[n])
            cnts[si] = d
            snode[si] = n
            cap -= d
            si += 1
            n += 1
        if cap > 0 and n < nhi:
            # split head into slot 7 (tail continues next block slot 0)
            d = int(deg[n])
            take = min(d, cap)
            if e_start is None:
                e_start = int(first_edge[n])
            cnts[NSLOT - 1] = take
            cap -= take
            carry = (n, int(first_edge[n]) + take, d - take)
            n += 1
        if e_start is None:
            e_start = int(first_edge[min(n, nhi - 1)])
        blocks.append((e_start, P - cap, cnts))
        slot_nodes.append(snode)
    return blocks, np.asarray(slot_nodes, np.int64)


def _chunk_plan(B):
    chs = []
    r = B
    while r > 0:
        c = min(CH, r)
        chs.append(c)
        r -= c
    plans = []
    for ci, ch in enumerate(chs):
        ngrp = (ch + PSG - 1) // PSG
        built, ship = [], []
        beng = []
        for g in range(ngrp):
            k0, k1 = g * PSG, min((g + 1) * PSG, ch)
            a = GASSIGN[g % len(GASSIGN)]
            if ci == 0 and SHIP_FIRST_CHUNK:
                a = "ship"
            if a == "ship":
                ship.extend(range(k0, k1))
            else:
                built.extend(range(k0, k1))
                beng.append((a, k0, k1))
        plans.append((ch, built, ship, beng))
    return chs, plans


def _build_core_inputs(blocks, B, ysw_e, senc_e, rb_e, dij_e, plans):
    """Build device DRAM arrays for one core.

    Returns sx [128, sum(ch*72+nbu*24)] bf16 (per-chunk [S | srb]),
    dijd [128, nship*128] bf16.
    """
    nb = len(blocks)
    eb = np.array([b[0] for b in blocks], np.int64)
    ne = np.array([b[1] for b in blocks], np.int64)
    cnts = np.array([b[2] for b in blocks], np.int64)  # [nb, 8]

    blk_of = np.repeat(np.arange(nb), ne)              # per packed edge
    row_of = np.arange(ne.sum()) - np.repeat(np.cumsum(ne) - ne, ne)
    edge_of = np.repeat(eb, ne) + row_of
    slot_of = np.concatenate([
        np.repeat(np.arange(NSLOT), cnts[k]) for k in range(nb)
    ]) if nb else np.zeros(0, np.int64)

    # S: m-major [B, P, 9, 8]
    S = np.zeros((B, P, M9, NSLOT), np.float32)
    S[blk_of, row_of, :, slot_of] = ysw_e[edge_of]

    SRB = np.zeros((B, P, N_SPEC + N_RADIAL), np.float32)
    SRB[blk_of, row_of, :N_SPEC] = senc_e[edge_of]
    SRB[blk_of, row_of, N_SPEC:] = rb_e[edge_of]

    sx_parts = []
    ship_idx = []
    c0 = 0
    for ch, built, ship, _ in plans:
        sp = np.ascontiguousarray(
            S[c0:c0 + ch].transpose(1, 0, 2, 3)).reshape(P, ch * SC)
        bp = np.ascontiguousarray(
            SRB[[c0 + k for k in built]].transpose(1, 0, 2)
        ).reshape(P, -1)
        sx_parts.append(sp)
        sx_parts.append(bp)
        ship_idx.extend(c0 + k for k in ship)
        c0 += ch
    sx = np.concatenate(sx_parts, axis=1).astype(BF16)

    ship_idx = np.array(ship_idx, np.int64)
    if len(ship_idx):
        D = np.zeros((len(ship_idx), P, NB), np.float32)
        mask = np.isin(blk_of, ship_idx)
        pos = np.full(B, -1, np.int64)
        pos[ship_idx] = np.arange(len(ship_idx))
        D[pos[blk_of[mask]], row_of[mask], :] = dij_e[edge_of[mask]]
        dijd = np.ascontiguousarray(
            D.transpose(1, 0, 2)).reshape(P, -1).astype(BF16)
    else:
        dijd = np.zeros((P, NB), BF16)
    return sx, dijd


def _perm_w(W):
    """Permute Dense weight rows from rs-order (r*16+s) to (s*8+r)."""
    W = np.asarray(W, np.float32)
    return np.ascontiguousarray(
        W.reshape(N_RADIAL, N_SPEC, -1).transpose(1, 0, 2).reshape(NB, -1)
    )


# ========================= device program =========================

def _build_program(B):
    import concourse.bacc as bacc
    import concourse.mybir as mybir
    import concourse.tile as tile
    from concourse.alu_op_type import AluOpType as alu

    fp32 = mybir.dt.float32
    bf16 = mybir.dt.bfloat16

    assert B % CH == 0, "B must be a multiple of CH for this schedule"
    chs, plans = _chunk_plan(B)
    nchunk = len(chs)
    # per-chunk offsets into sx / dijd (in columns / blocks)
    sx_off = [0]
    dij_off = [0]
    for ch, built, ship, _ in plans:
        sx_off.append(sx_off[-1] + ch * SC + len(built) * 24)
        dij_off.append(dij_off[-1] + len(ship))
    nship_tot = dij_off[-1]

    nc = bacc.Bacc("TRN2", target_bir_lowering=False, debug=False,
                   num_devices=NCORES)

    sx_d = nc.dram_tensor("sx", [P, sx_off[-1]], bf16,
                          kind="ExternalInput")
    dijd_d = nc.dram_tensor("dijd", [P, max(nship_tot, 1) * NB], bf16,
                            kind="ExternalInput")
    wx_d = nc.dram_tensor("wx", [P, 3 * NCHAN], bf16, kind="ExternalInput")
    wy_d = nc.dram_tensor("wy", [P, 3 * NCHAN], bf16, kind="ExternalInput")
    out_d = nc.dram_tensor("out", [P, nchunk * 1176], bf16,
                           kind="ExternalOutput")

    with tile.TileContext(nc) as tc:
        with (
            tc.tile_pool(name="const", bufs=1) as cpool,
            tc.tile_pool(name="chunk", bufs=3) as ckpool,
            tc.tile_pool(name="big", bufs=3) as bigpool,
            tc.tile_pool(name="work", bufs=3) as wkpool,
            tc.tile_pool(name="out", bufs=3) as opool,
            tc.tile_pool(name="ps_sc", bufs=2, space="PSUM") as pssc,
            tc.tile_pool(name="ps_xy", bufs=2, space="PSUM") as psxy,
        ):
            wx = cpool.tile([P, 3 * NCHAN], bf16, tag="wx")
            wy = cpool.tile([P, 3 * NCHAN], bf16, tag="wy")
            nc.sync.dma_start(out=wx[:], in_=wx_d[:])
            nc.sync.dma_start(out=wy[:], in_=wy_d[:])

            # keep the PE clock ramping while the first chunk DMA lands
            dum = cpool.tile([P, NCHAN], bf16, tag="dum")
            nc.vector.memset(dum[:], 0.0)
            psdum = psxy.tile([P, 1024], fp32, tag="ps")
            for _ in range(20):
                nc.tensor.matmul(out=psdum[:, 0:NCHAN], lhsT=dum[:],
                                 rhs=dum[:], start=True, stop=True)

            rtiles = {}   # ci -> rhoi sbuf tile
            dtiles = {}   # ci -> (dij, sx)
            ptiles = {}   # ci -> pl output/product tile

            def dma_part(ci):
                ch, built, ship, beng = plans[ci]
                dij = ckpool.tile([P, CH * NB], bf16, tag="dij")
                ss0 = ckpool.tile([P, TBLK * SC], bf16, tag="ss0")
                ss1 = ckpool.tile([P, TBLK * SC], bf16, tag="ss1")
                ss2 = ckpool.tile([P, TBLK * SC], bf16, tag="ss2")
                sb = ckpool.tile([P, CH * 24], bf16, tag="sb")
                sss = (ss0, ss1, ss2)
                dtiles[ci] = (dij, sss, sb)
                o = sx_off[ci]
                for u in range(NT):
                    nc.sync.dma_start(
                        out=sss[u][:],
                        in_=sx_d[:, o + u * TBLK * SC:
                                 o + (u + 1) * TBLK * SC])
                nbu = len(built)
                if nbu:
                    nc.sync.dma_start(
                        out=sb[:, 0:nbu * 24],
                        in_=sx_d[:, o + ch * SC:o + ch * SC + nbu * 24])
                nsh = len(ship)
                if nsh:
                    od = dij_off[ci]
                    k0 = ship[0]
                    # one transfer per supertile slice so the first
                    # scatter matmuls never wait on the whole chunk
                    for a in range(k0, k0 + nsh, 2 * PSG):
                        b = min(a + 2 * PSG, k0 + nsh)
                        oo = od + (a - k0)
                        nc.sync.dma_start(
                            out=dij[:, a * NB:b * NB],
                            in_=dijd_d[:, oo * NB:(oo + b - a) * NB])

            def build_part(ci):
                ch, built, ship, beng = plans[ci]
                dij, sss, sb = dtiles[ci]
                if not built:
                    return
                cpos = {k: i for i, k in enumerate(built)}
                dv = dij[:].rearrange("p (k s r) -> p k s r",
                                      s=N_SPEC, r=N_RADIAL)
                sv = sb[:, 0:len(built) * 24].rearrange(
                    "p (k c) -> p k c", c=24)
                for eng, k0, k1 in beng:
                    n = k1 - k0
                    i0 = cpos[k0]
                    senc = sv[:, i0:i0 + n, 0:N_SPEC]
                    rb = sv[:, i0:i0 + n, N_SPEC:24]
                    out = dv[:, k0:k1]
                    in0 = senc.unsqueeze(3).broadcast_to(
                        [P, n, N_SPEC, N_RADIAL])
                    in1 = rb.unsqueeze(2).broadcast_to(
                        [P, n, N_SPEC, N_RADIAL])
                    nc.gpsimd.tensor_tensor(out=out, in0=in0, in1=in1,
                                            op=alu.mult)

            def scatter_super(ci, sup):
                ch = chs[ci]
                dij, sss, sb = dtiles[ci]
                rhoi = rtiles[ci]
                pst = pssc.tile([P, 1024], fp32, tag="psc")
                kbase = sup * 2 * PSG
                nblk = min(2 * PSG, ch - kbase)
                ss = sss[sup]
                for j in range(nblk):
                    k = kbase + j
                    colb = (j // PSG) * 512 + (j % PSG) * SC
                    nc.tensor.matmul(
                        out=pst[:, colb:colb + SC],
                        lhsT=dij[:, k * NB:(k + 1) * NB],
                        rhs=ss[:, j * SC:(j + 1) * SC],
                        start=True, stop=True,
                    )
                assert nblk == 2 * PSG
                eng = RHO_COPY[sup % len(RHO_COPY)]
                src = pst[:].rearrange("p (g q) -> p g q", g=2)[
                    :, :, 0:PSG * SC].rearrange(
                    "p g (k c) -> p g k c", c=SC)
                dst = rhoi[:, kbase * SC:(kbase + nblk) * SC].rearrange(
                    "p (g k c) -> p g k c", g=2, c=SC)
                if eng == "act":
                    nc.scalar.copy(out=dst, in_=src)
                else:
                    nc.vector.tensor_copy(out=dst, in_=src)

            def merges(ci):
                ch = chs[ci]
                rv = rtiles[ci][:].rearrange("p (k m sl) -> p k m sl",
                                             m=M9, sl=NSLOT)
                nc.vector.tensor_tensor(
                    out=rv[:, 1:ch, :, 0],
                    in0=rv[:, 1:ch, :, 0],
                    in1=rv[:, 0:ch - 1, :, 7],
                    op=alu.add,
                )
                if ci > 0:
                    pch = chs[ci - 1]
                    prv = rtiles[ci - 1][:].rearrange(
                        "p (k m sl) -> p k m sl", m=M9, sl=NSLOT)
                    nc.vector.tensor_tensor(
                        out=rv[:, 0:1, :, 0],
                        in0=rv[:, 0:1, :, 0],
                        in1=prv[:, pch - 1:pch, :, 7],
                        op=alu.add,
                    )

            def phase3_tile(ci, t):
                rv = rtiles[ci][:].rearrange("p (k m sl) -> p k m sl",
                                             m=M9, sl=NSLOT)
                kk = t * TBLK
                ns = TBLK * 7  # 98
                pl = ptiles[ci]
                for unit in range(2):  # 0: l0+l1, 1: l2
                    ps = psxy.tile([P, 1024], fp32, tag="ps")
                    if unit == 0:
                        mms = [(0, 0, 0), (1, 0, 98), (1, 1, 196),
                               (1, 2, 294)]
                    else:
                        mms = [(2, mi, mi * 98) for mi in range(5)]
                    for off, w in ((0, wx), (512, wy)):
                        for l, mi, col in mms:
                            wl = w[:, l * NCHAN:(l + 1) * NCHAN]
                            mov = rv[:, kk:kk + TBLK, l * l + mi, 0:7]
                            nc.tensor.matmul(
                                out=ps[:, off + col:off + col + ns],
                                lhsT=wl, rhs=mov, start=True, stop=True)
                    nx = 392 if unit == 0 else 490
                    ysb = wkpool.tile([P, 512], bf16, tag="ysb")
                    nc.scalar.copy(out=ysb[:, 0:nx],
                                   in_=ps[:, 512:512 + nx])
                    if unit == 0:
                        nc.vector.tensor_tensor(
                            out=pl[:, PRO + t * 882:PRO + t * 882 + 392],
                            in0=ps[:, 0:392], in1=ysb[:, 0:392],
                            op=alu.mult)
                    else:
                        nc.vector.tensor_tensor(
                            out=pl[:, PRO + t * 882 + 392:
                                   PRO + t * 882 + 882],
                            in0=ps[:, 0:490], in1=ysb[:, 0:490],
                            op=alu.mult)

            def chunk_tail(ci):
                # chunk-batched m-sums + r0 extract + single output DMA
                ch = chs[ci]
                nt = ch // TBLK
                pl = ptiles.pop(ci)
                rv = rtiles[ci][:].rearrange("p (k m sl) -> p k m sl",
                                             m=M9, sl=NSLOT)
                adde = ADDS_ENG[ci % len(ADDS_ENG)]
                ae = nc.vector if adde == "dve" else nc.gpsimd
                pv = pl[:, PRO:PRO + nt * 882].rearrange(
                    "p (t w) -> p t w", w=882)
                ov = pl[:, ODO:ODO + nt * 294].rearrange(
                    "p (t l s) -> p t l s", l=3, s=98)
                tmp = wkpool.tile([P, NT * 2 * 98], bf16, tag="tmp")
                t1 = tmp[:, 0:nt * 98].rearrange("p (t s) -> p t s", s=98)
                # l0 output: straight copy of the l0 products (4x mode)
                nc.vector.tensor_copy(out=ov[:, :, 0, :],
                                      in_=pv[:, :, 0:98])
                # l1: m0 + m1 + m2
                ae.tensor_tensor(out=t1, in0=pv[:, :, 196:294],
                                 in1=pv[:, :, 294:392], op=alu.add)
                ae.tensor_tensor(out=ov[:, :, 1, :], in0=t1,
                                 in1=pv[:, :, 98:196], op=alu.add)
                # l2: ((m1+m2)+(m3+m4)) + m0
                p2 = pv[:, :, 490:882].rearrange("p t (i s) -> p t i s",
                                                 i=2, s=196)
                t2 = tmp[:, 0:nt * 2 * 98].rearrange(
                    "p (t i s) -> p t i s", i=2, s=98)
                ae.tensor_tensor(out=t2, in0=p2[:, :, :, 0:98],
                                 in1=p2[:, :, :, 98:196], op=alu.add)
                ae.tensor_tensor(out=t1, in0=t2[:, :, 0, :],
                                 in1=t2[:, :, 1, :], op=alu.add)
                ae.tensor_tensor(out=ov[:, :, 2, :], in0=t1,
                                 in1=pv[:, :, 392:490], op=alu.add)
                # r0 = m=0 plane, slots 0..6
                nc.vector.tensor_copy(
                    out=pl[:, 0:ch * 7].rearrange("p (k s) -> p k s", s=7),
                    in_=rv[:, 0:ch, 0, 0:7],
                )
                nc.sync.dma_start(
                    out=out_d[:, ci * 1176:(ci + 1) * 1176],
                    in_=pl[:, 0:1176])

            # software pipeline: input DMA two chunks ahead; pool builds
            # one chunk ahead of scatter; phase-3 one chunk behind
            # scatter, interleaved supertile-by-tile on the PE queue.
            for ci in range(nchunk + 2):
                if ci < nchunk:
                    dma_part(ci)
                    build_part(ci)   # a full iteration before its scatter
                cs = ci - 1           # scatter chunk
                cp = ci - 2           # phase-3 chunk
                if 0 <= cs < nchunk:
                    rh = bigpool.tile([P, CH * SC], bf16, tag="rhoi")
                    rtiles[cs] = rh
                if 0 <= cp < nchunk:
                    plt = opool.tile([P, PLW], bf16, tag="pl")
                    ptiles[cp] = plt
                for u in range(NT):
                    if 0 <= cs < nchunk:
                        scatter_super(cs, u)
                    if 0 <= cp < nchunk:
                        phase3_tile(cp, u)
                if 0 <= cs < nchunk:
                    merges(cs)
                if 0 <= cp < nchunk:
                    chunk_tail(cp)

    nc.finalize()
    return nc


# ============================ entry point ============================

def kernel(**inputs):
    from concourse.bass_utils import run_bass_kernel_spmd

    dist = np.asarray(inputs["distances"], np.float32)
    vec = np.asarray(inputs["vec"], np.float32)
    switch = np.asarray(inputs["switch"], np.float32)
    st = np.asarray(inputs["species_table"], np.float32)
    species = np.asarray(inputs["species"], np.int64)
    esrc = np.asarray(inputs["edge_src"], np.int64)
    edst = np.asarray(inputs["edge_dst"], np.int64)
    N_NODES = species.shape[0]
    E = esrc.shape[0]

    deg = np.bincount(esrc, minlength=N_NODES)
    assert deg.max() <= P, "node degree exceeds 128"
    first_edge = np.searchsorted(esrc, np.arange(N_NODES + 1), side="left")
    splits = _partition_cores(esrc, N_NODES)

    # per-edge factors
    nvec = np.arange(1, N_RADIAL + 1, dtype=np.float32)
    rb_e = (np.sqrt(2.0 / CUTOFF) * np.sin(nvec[None, :] * (np.pi / CUTOFF)
                                           * dist[:, None]) / dist[:, None]
            * switch[:, None]).astype(np.float32)           # [E, 8]
    senc_e = st[species[edst]]                              # [E, 16]
    u = vec / dist[:, None]
    x, y, z = u[:, 0], u[:, 1], u[:, 2]
    ysw_e = (np.stack([
        np.ones_like(x), x, y, z, x * y, y * z,
        3.0 * z * z - 1.0, x * z, x * x - y * y,
    ], axis=-1) * KM[None, :]).astype(np.float32)

    cores = []
    maxb = 0
    for c in range(NCORES):
        blocks, slot_node = _pack_core(deg, first_edge,
                                       splits[c], splits[c + 1])
        cores.append((blocks, slot_node))
        maxb = max(maxb, len(blocks))
    B = ((maxb + CH - 1) // CH) * CH
    B7 = B * (NSLOT - 1)
    chs, plans = _chunk_plan(B)
    nchunk = len(chs)

    # dij for shipped blocks: match device build numerics (bf16 inputs)
    dij_e = (senc_e.astype(BF16).astype(np.float32)[:, :, None]
             * rb_e.astype(BF16).astype(np.float32)[:, None, :]
             ).reshape(E, NB)

    wx = np.empty((P, 3 * NCHAN), np.float32)
    wy = np.empty((P, 3 * NCHAN), np.float32)
    for l, key in enumerate(("W0", "W1", "W2")):
        Wp = _perm_w(inputs[key])
        wx[:, l * NCHAN:(l + 1) * NCHAN] = Wp[:, :NCHAN]
        wy[:, l * NCHAN:(l + 1) * NCHAN] = (
            Wp[:, NCHAN:] / np.sqrt(2 * l + 1.0))
    wx = wx.astype(BF16)
    wy = wy.astype(BF16)

    in_maps = []
    for c in range(NCORES):
        blocks, _ = cores[c]
        sx, dijd = _build_core_inputs(
            blocks, B, ysw_e, senc_e, rb_e, dij_e, plans)
        in_maps.append({"sx": sx, "dijd": dijd, "wx": wx, "wy": wy})

    if B not in _COMPILED:
        _COMPILED[B] = _build_program(B)
    nc = _COMPILED[B]

    res = run_bass_kernel_spmd(nc, in_maps, list(range(NCORES)),
                               trace=TRACE)
    global LAST_RESULT
    LAST_RESULT = res

    # ---------------- host assembly ----------------
    out = np.zeros((N_NODES, N_SPEC + NB + 3 * NCHAN), np.float32)
    out[:, :N_SPEC] = st[species]

    # device basis row of original index rs = r*16+s is dev = s*8+r
    r = np.arange(NB) // N_SPEC
    sidx = np.arange(NB) % N_SPEC
    dev_of_rs = sidx * N_RADIAL + r

    for c in range(NCORES):
        _, slot_node = cores[c]
        sn = np.full((B, NSLOT - 1), -1, np.int64)
        sn[:slot_node.shape[0]] = slot_node[:, :NSLOT - 1]
        sn = sn.reshape(-1)
        valid = sn >= 0
        nodes = sn[valid]
        slots = np.nonzero(valid)[0]
        o = np.asarray(res.results[c]["out"], np.float32)  # [128, nck*1176]
        ov = o.reshape(P, nchunk, 1176)
        r0 = ov[:, :, 0:294].reshape(P, B7)
        xyv = ov[:, :, 294:1176].reshape(P, nchunk, NT, 3, 98)
        out[nodes, N_SPEC:N_SPEC + NB] = r0[dev_of_rs][:, slots].T
        for l in range(3):
            xy_l = np.ascontiguousarray(
                xyv[:, :, :, l, :]).reshape(P, B7)
            out[nodes,
                N_SPEC + NB + l * NCHAN:N_SPEC + NB + (l + 1) * NCHAN] = (
                xy_l[:, slots].T)
    return out


# revision 33
# speedup vs baseline: 1.1628x; 1.0649x over previous
"""Trainium2 Bass kernel for FOAM embedding (GNN message passing).

Strategy (8 NeuronCores, SPMD, no collectives):
  - Edges are sorted by edge_src. Host partitions nodes into 8 contiguous
    ranges with balanced edge counts; each core owns its nodes' edges.
  - Host packs edges into blocks of EXACTLY 128 edges (the SBUF
    partitions). Each block has 8 node slots: slots 0..6 hold completed
    nodes, slot 7 holds the head of a node split at the 128-edge
    boundary; its tail continues in slot 0 of the next block. A single
    strided DVE add merges slot-7 partials into the next block's slot 0.
  - Host precomputes, per edge, Dij = senc[dst] (x) (bessel*switch)
    [128 basis cols] and S = onehot(slot) (x) (Y*km) [9m x 8slot cols],
    ships both as bf16. The device is a pure matmul pipeline:
      scatter:  PSUM[basis, (m,slot)] = Dij^T @ S      per block
      phase 3:  x = WxT rho_m, y = WyT rho_m per l; out = sum_m x*y
  - Outputs rhoi0 (m=0 plane) and xy per (l, slot) in bf16; host
    reassembles the full [15000, 528] fp32 output.
"""

import os
import sys

import numpy as np

for _p in ("/opt/trn_rl_repo", "/root/.axon_site/_ro/trn_rl_repo"):
    if os.path.isdir(_p) and _p not in sys.path:
        sys.path.insert(0, _p)

import ml_dtypes  # noqa: E402

# ---------------- problem constants (hardcoded per spec) ----------------
N_RADIAL = 8
N_SPEC = 16
ZMAX = 64
CUTOFF = 5.0
NCHAN = 128
NB = N_RADIAL * N_SPEC  # 128 basis
M9 = 9                  # real SH components up to l=2

NCORES = 8
P = 128                 # edges per block == partitions
NSLOT = 8               # 7 completed-node slots + 1 split-head slot
BCOL = M9 * NSLOT       # 72 S columns per block (m-outer: col = m*8+s)
TBLK = 14               # blocks per phase-3 tile (5m*14*7 = 490 <= 512)
CH = 42                 # blocks per chunk (3 phase-3 tiles)
PSG = 7                 # blocks per scatter PSUM tile (7*72 = 504)

BF16 = ml_dtypes.bfloat16

_COMPILED = {}
TRACE = False          # set True to capture an NTFF profile
LAST_RESULT = None     # BassKernelResults of the last kernel() call

_S3, _S5, _S15 = 3.0 ** 0.5, 5.0 ** 0.5, 15.0 ** 0.5
KM = np.array([1.0, _S3, _S3, _S3, _S15, _S15,
               0.5 * _S5, _S15, 0.5 * _S15], np.float32)


# ======================= host-side preprocessing =======================

def _partition_cores(edge_src, n_nodes):
    """Split nodes into NCORES contiguous ranges with ~equal edges."""
    es = np.asarray(edge_src, dtype=np.int64)
    E = es.shape[0]
    splits = [0]
    for c in range(1, NCORES):
        n = int(es[min((c * E) // NCORES, E - 1)])
        n = max(n, splits[-1])
        splits.append(n)
    splits.append(n_nodes)
    return splits


def _pack_core(deg, first_edge, nlo, nhi):
    """Pack nodes [nlo, nhi) into exact-128-edge blocks.

    Returns (blocks, slot_node) where blocks is a list of
    (e_start, n_edges, cnts[8]) and slot_node is [nblk, 8] node ids
    for completed slots (slots 0..6; -1 elsewhere).
    """
    blocks = []
    slot_nodes = []
    n = nlo
    carry = None  # (node, e_start, cnt) continuation -> slot 0
    while n < nhi or carry is not None:
        cnts = [0] * NSLOT
        snode = [-1] * NSLOT
        cap = P
        e_start = None
        si = 0
        if carry is not None:
            node, es0, cnt = carry
            assert cnt <= cap, f"node {node} degree too large"
            e_start = es0
            cnts[0] = cnt
            snode[0] = node
            cap -= cnt
            si = 1
            carry = None
        while n < nhi and si < NSLOT - 1:
            d = int(deg[n])
            if d > cap:
                break
            if e_start is None:
                e_start = int(first_edge[n])
            cnts[si] = d
            snode[si] = n
            cap -= d
            si += 1
            n += 1
        if cap > 0 and n < nhi:
            # split head into slot 7 (tail continues next block slot 0)
            d = int(deg[n])
            take = min(d, cap)
            if e_start is None:
                e_start = int(first_edge[n])
            cnts[NSLOT - 1] = take
            cap -= take
            carry = (n, int(first_edge[n]) + take, d - take)
            n += 1
        if e_start is None:
            e_start = int(first_edge[min(n, nhi - 1)])
        blocks.append((e_start, P - cap, cnts))
        slot_nodes.append(snode)
    return blocks, np.asarray(slot_nodes, np.int64)


def _build_core_inputs(blocks, B, dij_e, ysw_e):
    """Build device DRAM arrays for one core.

    dij_e: [E, 128] fp32 per-edge Dij rows (global edge indexing)
    ysw_e: [E, 9] fp32 per-edge Y*km rows
    Returns dij [128, B*128] bf16, s [128, B*72] bf16.
    """
    nb = len(blocks)
    eb = np.array([b[0] for b in blocks], np.int64)
    ne = np.array([b[1] for b in blocks], np.int64)
    cnts = np.array([b[2] for b in blocks], np.int64)  # [nb, 8]

    blk_of = np.repeat(np.arange(nb), ne)              # per packed edge
    row_of = np.arange(ne.sum()) - np.repeat(np.cumsum(ne) - ne, ne)
    edge_of = np.repeat(eb, ne) + row_of
    slot_of = np.concatenate([
        np.repeat(np.arange(NSLOT), cnts[k]) for k in range(nb)
    ]) if nb else np.zeros(0, np.int64)

    D = np.zeros((B, P, NB), np.float32)
    D[blk_of, row_of, :] = dij_e[edge_of]
    S = np.zeros((B, P, M9, NSLOT), np.float32)
    S[blk_of, row_of, :, slot_of] = ysw_e[edge_of]

    dij = np.ascontiguousarray(D.transpose(1, 0, 2)).reshape(P, B * NB)
    s = np.ascontiguousarray(S.transpose(1, 0, 2, 3)).reshape(P, B * BCOL)
    return dij.astype(BF16), s.astype(BF16)


def _perm_w(W):
    """Permute Dense weight rows from rs-order (r*16+s) to (s*8+r)."""
    W = np.asarray(W, np.float32)
    return np.ascontiguousarray(
        W.reshape(N_RADIAL, N_SPEC, -1).transpose(1, 0, 2).reshape(NB, -1)
    )


# ========================= device program =========================

def _build_program(B):
    import concourse.bacc as bacc
    import concourse.mybir as mybir
    import concourse.tile as tile
    from concourse.alu_op_type import AluOpType as alu

    fp32 = mybir.dt.float32
    bf16 = mybir.dt.bfloat16

    assert B % TBLK == 0
    chs = []
    r = B
    while r > 0:
        c = min(CH, r)
        chs.append(c)
        r -= c
    cstart = np.cumsum([0] + chs).tolist()
    B7 = B * (NSLOT - 1)  # output slots per l

    nc = bacc.Bacc("TRN2", target_bir_lowering=False, debug=False,
                   num_devices=NCORES)

    dij_d = nc.dram_tensor("dij", [P, B * NB], bf16, kind="ExternalInput")
    s_d = nc.dram_tensor("s", [P, B * BCOL], bf16, kind="ExternalInput")
    wx_d = nc.dram_tensor("wx", [P, 3 * NCHAN], bf16, kind="ExternalInput")
    wy_d = nc.dram_tensor("wy", [P, 3 * NCHAN], bf16, kind="ExternalInput")
    r0_d = nc.dram_tensor("rhoi0", [P, B7], bf16, kind="ExternalOutput")
    xy_d = nc.dram_tensor("xy", [P, 3 * B7], bf16, kind="ExternalOutput")

    with tile.TileContext(nc) as tc:
        with (
            tc.tile_pool(name="const", bufs=1) as cpool,
            tc.tile_pool(name="chunk", bufs=3) as ckpool,
            tc.tile_pool(name="big", bufs=3) as bigpool,
            tc.tile_pool(name="work", bufs=2) as wkpool,
            tc.tile_pool(name="ps_sc", bufs=3, space="PSUM") as pssc,
            tc.tile_pool(name="ps_x", bufs=2, space="PSUM") as psx,
            tc.tile_pool(name="ps_y", bufs=2, space="PSUM") as psy,
        ):
            wx = cpool.tile([P, 3 * NCHAN], bf16, tag="wx")
            wy = cpool.tile([P, 3 * NCHAN], bf16, tag="wy")
            nc.sync.dma_start(out=wx[:], in_=wx_d[:])
            nc.sync.dma_start(out=wy[:], in_=wy_d[:])

            # HAM warm-up primer: ~4us of back-to-back dummy matmuls
            # while the first chunk DMAs land, so the PE clock gate is
            # at 2.4 GHz (K=8/8) before real work starts.
            dum = cpool.tile([P, NCHAN], bf16, tag="dum")
            nc.vector.memset(dum[:], 0.0)
            psdum = psx.tile([P, 512], fp32, tag="xp")
            for _ in range(30):
                nc.tensor.matmul(out=psdum[:, 0:NCHAN], lhsT=dum[:],
                                 rhs=dum[:], start=True, stop=True)

            # per-chunk rhoi tiles (pool) so phase 3 of chunk c has no
            # false dependency on chunk c+1's writes
            rtiles = {}
            dtiles = {}

            def dma_part(ci):
                # issue chunk input DMAs two iterations ahead, split in
                # halves so the first matmuls of the chunk start after
                # half of the transfer instead of all of it
                ch = chs[ci]
                c0 = cstart[ci]
                h = ch // 2
                dija = ckpool.tile([P, (CH // 2) * NB], bf16, tag="dija")
                dijb = ckpool.tile([P, (CH - CH // 2) * NB], bf16,
                                   tag="dijb")
                sa = ckpool.tile([P, (CH // 2) * BCOL], bf16, tag="sa")
                sb = ckpool.tile([P, (CH - CH // 2) * BCOL], bf16,
                                 tag="sb")
                dtiles[ci] = (dija, dijb, sa, sb, h)
                nc.sync.dma_start(
                    out=dija[:, 0:h * NB],
                    in_=dij_d[:, c0 * NB:(c0 + h) * NB])
                nc.sync.dma_start(
                    out=sa[:, 0:h * BCOL],
                    in_=s_d[:, c0 * BCOL:(c0 + h) * BCOL])
                nc.sync.dma_start(
                    out=dijb[:, 0:(ch - h) * NB],
                    in_=dij_d[:, (c0 + h) * NB:(c0 + ch) * NB])
                nc.sync.dma_start(
                    out=sb[:, 0:(ch - h) * BCOL],
                    in_=s_d[:, (c0 + h) * BCOL:(c0 + ch) * BCOL])

            def scatter_part(ci):
                ch = chs[ci]
                c0 = cstart[ci]
                dija, dijb, sa, sb, h = dtiles.pop(ci)

                def dij_ap(k):
                    return (dija[:, k * NB:(k + 1) * NB] if k < h else
                            dijb[:, (k - h) * NB:(k - h + 1) * NB])

                def s_ap(k):
                    return (sa[:, k * BCOL:(k + 1) * BCOL] if k < h else
                            sb[:, (k - h) * BCOL:(k - h + 1) * BCOL])

                rhoi = bigpool.tile([P, CH * BCOL], bf16, tag="rhoi")
                rtiles[ci] = rhoi
                rv = rhoi[:].rearrange("p (k m s) -> p k m s",
                                       m=M9, s=NSLOT)

                # segment-sum via per-block matmuls; merges emitted
                # per group right after each copy so phase 3 never
                # waits on a whole-chunk merge
                for g in range(ch // PSG):
                    pst = pssc.tile([P, PSG * BCOL], fp32, tag="psc")
                    for j in range(PSG):
                        k = g * PSG + j
                        nc.tensor.matmul(
                            out=pst[:, j * BCOL:(j + 1) * BCOL],
                            lhsT=dij_ap(k),
                            rhs=s_ap(k),
                            start=True, stop=True,
                        )
                    col0 = g * PSG * BCOL
                    dst = rhoi[:, col0:col0 + PSG * BCOL]
                    if g % 6 == 5:
                        nc.vector.tensor_copy(out=dst, in_=pst[:])
                    else:
                        nc.scalar.copy(out=dst, in_=pst[:])
                    # merge split-node partials slot7[k-1] -> slot0[k]
                    # for this group's blocks
                    k0 = g * PSG if g > 0 else 1
                    k1 = (g + 1) * PSG
                    nc.gpsimd.tensor_tensor(
                        out=rv[:, k0:k1, :, 0],
                        in0=rv[:, k0:k1, :, 0],
                        in1=rv[:, k0 - 1:k1 - 1, :, 7],
                        op=alu.add,
                    )
                    if g == 0 and ci > 0:
                        # boundary with previous chunk's last block
                        pch = chs[ci - 1]
                        prv = rtiles[ci - 1][:].rearrange(
                            "p (k m s) -> p k m s", m=M9, s=NSLOT)
                        nc.gpsimd.tensor_tensor(
                            out=rv[:, 0:1, :, 0],
                            in0=rv[:, 0:1, :, 0],
                            in1=prv[:, pch - 1:pch, :, 7],
                            op=alu.add,
                        )

                # rhoi0 output (m=0 plane, slots 0..6)
                r0t = wkpool.tile([P, CH * 7], bf16, tag="r0t")
                nc.gpsimd.tensor_copy(
                    out=r0t[:, 0:ch * 7],
                    in_=rv[:, 0:ch, 0, 0:7],
                )
                nc.sync.dma_start(out=r0_d[:, c0 * 7:(c0 + ch) * 7],
                                  in_=r0t[:, 0:ch * 7])

            def phase3_part(ci):
                ch = chs[ci]
                c0 = cstart[ci]
                ntile = ch // TBLK
                rv = rtiles[ci][:].rearrange("p (k m s) -> p k m s",
                                             m=M9, s=NSLOT)
                for l in range(3):
                    mg = 2 * l + 1
                    m0 = l * l
                    wxl = wx[:, l * NCHAN:(l + 1) * NCHAN]
                    wyl = wy[:, l * NCHAN:(l + 1) * NCHAN]
                    ol = wkpool.tile([P, CH * 7], bf16, tag=f"ol{l}")
                    ov = ol[:, 0:ch * 7].rearrange(
                        "p (t s) -> p t s", s=98)
                    pl = wkpool.tile([P, (CH // TBLK) * 5 * 98], bf16,
                                     tag=f"pl{l}")
                    for t in range(ntile):
                        kk = t * TBLK
                        xp = psx.tile([P, 512], fp32, tag="xp")
                        yp = psy.tile([P, 512], fp32, tag="yp")
                        for mi in range(mg):
                            mov = rv[:, kk:kk + TBLK, m0 + mi, 0:7]
                            nc.tensor.matmul(
                                out=xp[:, mi * 98:(mi + 1) * 98],
                                lhsT=wxl, rhs=mov, start=True, stop=True)
                            nc.tensor.matmul(
                                out=yp[:, mi * 98:(mi + 1) * 98],
                                lhsT=wyl, rhs=mov, start=True, stop=True)
                        pdst = (ol[:, t * 98:(t + 1) * 98] if l == 0 else
                                pl[:, t * mg * 98:(t + 1) * mg * 98])
                        # TT may read at most one PSUM operand: stage y
                        # through SBUF (scalar), multiply on DVE.
                        ysb = wkpool.tile([P, 512], bf16, tag="ysb")
                        if l == 0:
                            nc.vector.tensor_copy(out=ysb[:, 0:mg * 98],
                                                  in_=yp[:, 0:mg * 98])
                        else:
                            nc.scalar.copy(out=ysb[:, 0:mg * 98],
                                           in_=yp[:, 0:mg * 98])
                        nc.vector.tensor_tensor(
                            out=pdst,
                            in0=xp[:, 0:mg * 98], in1=ysb[:, 0:mg * 98],
                            op=alu.mult,
                        )
                        # sum over m per tile: small contiguous DVE adds
                        # (gpsimd pays ~250ns fixed cost per op, DVE
                        # ~60ns -- keep gpsimd for merges/r0 only)
                        if l == 0:
                            continue
                        pt = pdst.rearrange("p (m s) -> p m s", s=98)
                        od = ol[:, t * 98:(t + 1) * 98]
                        if l == 1:
                            tmp = wkpool.tile([P, 128], bf16, tag="tmp1")
                            nc.vector.tensor_tensor(
                                out=tmp[:, 0:98], in0=pt[:, 0, :],
                                in1=pt[:, 1, :], op=alu.add)
                            nc.vector.tensor_tensor(
                                out=od, in0=tmp[:, 0:98],
                                in1=pt[:, 2, :], op=alu.add)
                    if l == 2:
                        # l2 adds batched per chunk on gpsimd (4 big ops)
                        pv = pl[:, 0:ntile * 5 * 98].rearrange(
                            "p (t m s) -> p t m s", m=5, s=98)
                        tmpa = wkpool.tile([P, CH * 7], bf16, tag="tmp2a")
                        tmpb = wkpool.tile([P, CH * 7], bf16, tag="tmp2b")
                        tva = tmpa[:, 0:ch * 7].rearrange(
                            "p (t s) -> p t s", s=98)
                        tvb = tmpb[:, 0:ch * 7].rearrange(
                            "p (t s) -> p t s", s=98)
                        nc.gpsimd.tensor_tensor(
                            out=tva, in0=pv[:, :, 0, :], in1=pv[:, :, 1, :],
                            op=alu.add)
                        nc.gpsimd.tensor_tensor(
                            out=tvb, in0=pv[:, :, 2, :], in1=pv[:, :, 3, :],
                            op=alu.add)
                        nc.gpsimd.tensor_tensor(
                            out=tva, in0=tva, in1=tvb, op=alu.add)
                        nc.gpsimd.tensor_tensor(
                            out=ov, in0=tva, in1=pv[:, :, 4, :], op=alu.add)
                    nc.sync.dma_start(
                        out=xy_d[:, l * B7 + c0 * 7:l * B7 + (c0 + ch) * 7],
                        in_=ol[:, 0:ch * 7])

            # software pipeline: input DMA runs two iterations ahead of
            # its scatter; phase 3 runs one chunk behind scatter so the
            # PE never stalls on the copy->merge chain.
            nchunk = len(chs)
            for ci in range(nchunk + 2):
                if ci < nchunk:
                    dma_part(ci)
                if 1 <= ci <= nchunk:
                    scatter_part(ci - 1)
                if ci >= 2:
                    phase3_part(ci - 2)

    nc.finalize()
    return nc


# ============================ entry point ============================

def kernel(**inputs):
    from concourse.bass_utils import run_bass_kernel_spmd

    dist = np.asarray(inputs["distances"], np.float32)
    vec = np.asarray(inputs["vec"], np.float32)
    switch = np.asarray(inputs["switch"], np.float32)
    st = np.asarray(inputs["species_table"], np.float32)
    species = np.asarray(inputs["species"], np.int64)
    esrc = np.asarray(inputs["edge_src"], np.int64)
    edst = np.asarray(inputs["edge_dst"], np.int64)
    N_NODES = species.shape[0]
    E = esrc.shape[0]

    deg = np.bincount(esrc, minlength=N_NODES)
    assert deg.max() <= P, "node degree exceeds 128"
    first_edge = np.searchsorted(esrc, np.arange(N_NODES + 1), side="left")
    splits = _partition_cores(esrc, N_NODES)

    # per-edge factors
    nvec = np.arange(1, N_RADIAL + 1, dtype=np.float32)
    rb = (np.sqrt(2.0 / CUTOFF) * np.sin(nvec[None, :] * (np.pi / CUTOFF)
                                         * dist[:, None]) / dist[:, None]
          * switch[:, None]).astype(np.float32)            # [E, 8]
    senc_e = st[species[edst]]                             # [E, 16]
    dij_e = (senc_e[:, :, None] * rb[:, None, :]).reshape(E, NB)
    u = vec / dist[:, None]
    x, y, z = u[:, 0], u[:, 1], u[:, 2]
    ysw_e = (np.stack([
        np.ones_like(x), x, y, z, x * y, y * z,
        3.0 * z * z - 1.0, x * z, x * x - y * y,
    ], axis=-1) * KM[None, :]).astype(np.float32)

    cores = []
    maxb = 0
    for c in range(NCORES):
        blocks, slot_node = _pack_core(deg, first_edge,
                                       splits[c], splits[c + 1])
        cores.append((blocks, slot_node))
        maxb = max(maxb, len(blocks))
    B = ((maxb + TBLK - 1) // TBLK) * TBLK
    B7 = B * (NSLOT - 1)

    wx = np.empty((P, 3 * NCHAN), np.float32)
    wy = np.empty((P, 3 * NCHAN), np.float32)
    for l, key in enumerate(("W0", "W1", "W2")):
        Wp = _perm_w(inputs[key])
        wx[:, l * NCHAN:(l + 1) * NCHAN] = Wp[:, :NCHAN]
        wy[:, l * NCHAN:(l + 1) * NCHAN] = (
            Wp[:, NCHAN:] / np.sqrt(2 * l + 1.0))
    wx = wx.astype(BF16)
    wy = wy.astype(BF16)

    in_maps = []
    for c in range(NCORES):
        blocks, _ = cores[c]
        dij, s = _build_core_inputs(blocks, B, dij_e, ysw_e)
        in_maps.append({"dij": dij, "s": s, "wx": wx, "wy": wy})

    if B not in _COMPILED:
        _COMPILED[B] = _build_program(B)
    nc = _COMPILED[B]

    res = run_bass_kernel_spmd(nc, in_maps, list(range(NCORES)),
                               trace=TRACE)
    global LAST_RESULT
    LAST_RESULT = res

    # ---------------- host assembly ----------------
    out = np.zeros((N_NODES, N_SPEC + NB + 3 * NCHAN), np.float32)
    out[:, :N_SPEC] = st[species]

    # device basis row of original index rs = r*16+s is dev = s*8+r
    r = np.arange(NB) // N_SPEC
    sidx = np.arange(NB) % N_SPEC
    dev_of_rs = sidx * N_RADIAL + r

    for c in range(NCORES):
        _, slot_node = cores[c]
        sn = np.full((B, NSLOT - 1), -1, np.int64)
        sn[:slot_node.shape[0]] = slot_node[:, :NSLOT - 1]
        sn = sn.reshape(-1)
        valid = sn >= 0
        nodes = sn[valid]
        slots = np.nonzero(valid)[0]
        r0 = np.asarray(res.results[c]["rhoi0"], np.float32)  # [128, B7]
        xy = np.asarray(res.results[c]["xy"], np.float32)     # [128, 3*B7]
        out[nodes, N_SPEC:N_SPEC + NB] = r0[dev_of_rs][:, slots].T
        for l in range(3):
            out[nodes,
                N_SPEC + NB + l * NCHAN:N_SPEC + NB + (l + 1) * NCHAN] = (
                xy[:, l * B7 + slots].T)
    return out

